# revision 14
# baseline (speedup 1.0000x reference)
"""Trainium2 Bass kernel for nn_BL_36721970381090 (dense_mlp).

Math: the reference collapses to out[b] = M2 @ relu(M1 @ vec(x[b]) + b1) + b2,
M1 = perm(kron(W11, fc2_w)) [600, 400] (exact rank 200), M2 [3, 600].

Per 512-batch block (u-major intermediate g = 200 rows split 120+80):
  A (8 MMs): pg [128, 1024] psum (2 banks):
     bank0 = g0 (u in {0,1,2}) [120 r], bank1 = g1 (u in {3,4}) [80 r + 40 zero r]
     each bank accumulates 4 chunk-MMs contracting 100 x-rows.
  g-copy (1 op): pg[0:120, 0:1024] -> g_sb bf16. g_sb is a STATIC tile whose
     row 120 holds constant 1.0 (written once) -> bias1 is folded into stage B
     as contraction row 120 of the stationary.
  B (5 MMs): y chunk m = (t, u=m) -> y_super [128, 2560] psum (5 banks,
     chunk m in bank m); lhsT = bw2[0:121, 120m:...] (row 120 = bias1[:, m]).
  relu (1 op): y_super[0:120, 0:2560] -> r_super bf16 (no bias needed).
  C (5 MMs): col-tiled m=0..3 at tile_position (0, 32m) -> concurrent in one
     PE span; partials in pc bank partitions {0-2,32-34,64-66,96-98}; m=4
     accumulates onto the (0,0) slice (start=False).
  pc-copy (1 op) -> osb; 4 partial [3, *] slices summed on host.

Engine economy: 3 ACT/DVE ops per block (vs 12 naive) because engine cost is
free-size driven (~1ns/elem + ~130ns), partitions are free.
"""

import numpy as np
import ml_dtypes
from contextlib import ExitStack

import concourse.bass as bass
import concourse.bacc as bacc
import concourse.mybir as mybir
from concourse.bass import ds
from concourse.tile import TileContext
from concourse.bass_utils import run_bass_kernel_spmd

B, D1, D2 = 131072, 40, 10
T0, T1, O0 = 120, 5, 3
NCORES = 8
BC = B // NCORES          # 16384 batch per core
NB = 512                  # psum free-dim block
NBLK = BC // NB           # 32 blocks
XGRP = 1                  # blocks per x DMA
OGRP = 4                  # blocks per output DMA group

F32 = mybir.dt.float32
BF16 = mybir.dt.bfloat16
BF = ml_dtypes.bfloat16
RELU = mybir.ActivationFunctionType.Relu
COPY = mybir.ActivationFunctionType.Copy
ADD = mybir.AluOpType.add
MAX = mybir.AluOpType.max

COLTILE = True            # col-tiled stage C (4 concurrent MMs)

_CACHE = {}


def _build_nc():
    nc = bacc.Bacc()
    xt = nc.dram_tensor("xt", (100, NBLK * 4 * NB), BF16, kind="ExternalInput")
    aw = nc.dram_tensor("aw", (100, 1024), BF16, kind="ExternalInput")
    bw = nc.dram_tensor("bw", (121, 600), BF16, kind="ExternalInput")
    cw = nc.dram_tensor("cw", (120, 15), BF16, kind="ExternalInput")
    ones = nc.dram_tensor("ones", (1, 1024), BF16, kind="ExternalInput")
    outp = nc.dram_tensor("outp", (12, BC), F32, kind="ExternalOutput")

    with TileContext(nc) as tc, ExitStack() as ctx:
        consts = ctx.enter_context(tc.tile_pool(name="consts", bufs=1))
        aw_sb = consts.tile([100, 1024], BF16, tag="aw")
        bw_sb = consts.tile([121, 600], BF16, tag="bw")
        cw_sb = consts.tile([120, 15], BF16, tag="cw")
        # two static g tiles (separate ACT/DVE writers -> no serialization);
        # row 120 of each holds constant 1.0 (bias-fold contraction row)
        g0_sb = consts.tile([128, NB], BF16, tag="g0sb")
        g1_sb = consts.tile([128, NB], BF16, tag="g1sb")

        xpool = ctx.enter_context(tc.tile_pool(name="xp", bufs=3))
        rpool = ctx.enter_context(tc.tile_pool(name="rp", bufs=2))
        opool = ctx.enter_context(tc.tile_pool(name="op", bufs=2))
        pgpool = ctx.enter_context(tc.tile_pool(name="pg", bufs=1, space="PSUM"))
        pypool = ctx.enter_context(tc.tile_pool(name="py", bufs=1, space="PSUM"))
        pcpool = ctx.enter_context(tc.tile_pool(name="pc", bufs=1, space="PSUM"))

        xts = {}

        def issue_x_dma(jj):
            if jj % XGRP == 0 and jj < NBLK:
                xt2 = xpool.tile([100, XGRP * 4 * NB], BF16, tag="xt2",
                                 name=f"xt2_{jj}")
                nc.sync.dma_start(xt2[:, :],
                                  xt[:, ds(jj * 4 * NB, XGRP * 4 * NB)])
                xts[jj // XGRP] = xt2

        # x DMA first (A(0) critical path), then the small consts
        issue_x_dma(0)
        nc.sync.dma_start(aw_sb[:, :], aw[:, :])
        issue_x_dma(1)
        nc.sync.dma_start(bw_sb[:, :], bw[:, :])
        nc.sync.dma_start(cw_sb[:, :], cw[:, :])
        nc.sync.dma_start(g0_sb[120:121, :], ones[:, ds(0, NB)])
        nc.sync.dma_start(g1_sb[120:121, :], ones[:, ds(NB, NB)])

        def issue_A(jj):
            """DMA prefetch + 8 A-matmuls into a fresh pg tile. Returns pg."""
            issue_x_dma(jj + 2)
            xt2 = xts[jj // XGRP]
            xoff = (jj % XGRP) * 4 * NB
            pg = pgpool.tile([128, 2 * NB], F32, tag="pg", name=f"pg_{jj}")
            # g0 bank first so gcopy-a can fire mid-A
            for c in range(4):
                nc.tensor.matmul(pg[0:120, ds(0, NB)],
                                 aw_sb[:, ds(256 * c, 120)],
                                 xt2[:, ds(xoff + NB * c, NB)],
                                 start=(c == 0), stop=(c == 3))
            for c in range(4):
                nc.tensor.matmul(pg[0:120, ds(NB, NB)],
                                 aw_sb[:, ds(256 * c + 128, 120)],
                                 xt2[:, ds(xoff + NB * c, NB)],
                                 start=(c == 0), stop=(c == 3))
            return pg

        pg = issue_A(0)
        for jj in range(NBLK):
            # g-copies: bank0 on ACT, bank1 on DVE (separate dest tiles ->
            # independent writers, no serialization)
            nc.scalar.activation(g0_sb[0:120, :], pg[0:120, ds(0, NB)], COPY)
            nc.vector.tensor_copy(g1_sb[0:120, :], pg[0:120, ds(NB, NB)])

            # --- stage B: y_super [128, 2560] (chunk m in bank m) ---
            ys = pypool.tile([128, 5 * NB], F32, tag="ys", name=f"ys_{jj}")
            for m in range(5):
                gsb = g0_sb if m < 3 else g1_sb
                nc.tensor.matmul(ys[0:120, ds(NB * m, NB)],
                                 bw_sb[0:121, ds(120 * m, 120)],
                                 gsb[0:121, :],
                                 start=True, stop=True)
            # split relu into two tiles: chunks 0-3 on ACT, 4 on DVE
            r_a = rpool.tile([128, 4 * NB], BF16, tag="ra", name=f"ra_{jj}")
            r_b = rpool.tile([128, NB], BF16, tag="rb", name=f"rb_{jj}")
            nc.scalar.activation(r_a[0:120, :], ys[0:120, ds(0, 4 * NB)], RELU)
            nc.vector.tensor_scalar(r_b[0:120, :], ys[0:120, ds(4 * NB, NB)],
                                    0.0, None, op0=MAX)

            # next block's A-matmuls run while relu(jj) drains
            if jj + 1 < NBLK:
                pg = issue_A(jj + 1)

            # --- stage C ---
            pc = pcpool.tile([128, NB], F32, tag="pc", name=f"pc_{jj}")
            if COLTILE:
                for m in range(4):
                    nc.tensor.matmul(pc[32 * m:32 * m + 3, :],
                                     cw_sb[:, ds(3 * m, 3)],
                                     r_a[0:120, ds(NB * m, NB)],
                                     start=True, stop=(m != 0),
                                     tile_position=(0, 32 * m),
                                     skip_group_check=True)
                nc.tensor.matmul(pc[0:3, :], cw_sb[:, ds(12, 3)],
                                 r_b[0:120, :],
                                 start=False, stop=True, tile_position=(0, 0),
                                 skip_group_check=True)
            else:
                for m in range(5):
                    rt = r_a if m < 4 else r_b
                    roff = NB * m if m < 4 else 0
                    nc.tensor.matmul(pc[0:3, :], cw_sb[:, ds(3 * m, 3)],
                                     rt[0:120, ds(roff, NB)],
                                     start=(m == 0), stop=(m == 4))

            # --- drain C partials ---
            if jj % OGRP == 0:
                osb = opool.tile([128, OGRP * NB], F32, tag="osb",
                                 name=f"osb_{jj}")
            oslot = (jj % OGRP) * NB
            if COLTILE:
                nc.vector.tensor_copy(osb[0:99, ds(oslot, NB)], pc[0:99, :])
            else:
                nc.vector.tensor_copy(osb[0:3, ds(oslot, NB)], pc[0:3, :])
            if jj % OGRP == OGRP - 1:
                grp = jj // OGRP
                gslice = ds(grp * OGRP * NB, OGRP * NB)
                if COLTILE:
                    for m in range(4):
                        nc.sync.dma_start(outp[ds(3 * m, 3), gslice],
                                          osb[32 * m:32 * m + 3, :])
                else:
                    nc.sync.dma_start(outp[ds(0, 3), gslice], osb[0:3, :])
    nc.finalize()
    return nc


def _host_mats(W11, fc2_w, W12, fc4_w, bias1):
    """Build aw [100,1024], bw [121,600], cw [120,15] (fp32)."""
    aw = np.zeros((100, 1024), np.float32)
    for c in range(4):
        for p in range(100):
            d = 10 * c + p // 10
            s = p % 10
            for u in range(3):
                aw[p, 256 * c + u * 40 + d] = fc2_w[u, s]
            for u in (3, 4):
                aw[p, 256 * c + 128 + (u - 3) * 40 + d] = fc2_w[u, s]
    bw = np.zeros((121, 600), np.float32)
    for m in range(5):
        if m < 3:
            rows = slice(40 * m, 40 * m + 40)
        else:
            rows = slice(40 * (m - 3), 40 * (m - 3) + 40)
        bw[rows, 120 * m:120 * m + 120] = W11.T        # [40 d, 120 t]
        bw[120, 120 * m:120 * m + 120] = bias1[:, m]   # folded bias1
    M2 = np.kron(W12, fc4_w)                            # [3, 600]
    cw = np.zeros((120, 15), np.float32)
    for m in range(5):
        cw[:, 3 * m:3 * m + 3] = M2[:, m::5].T
    return aw, bw, cw


def kernel(x, W11, fc2_w, bias1, W12, fc4_w, bias2, _trace=False):
    x = np.asarray(x, dtype=np.float32)
    W11 = np.asarray(W11, np.float32)
    fc2_w = np.asarray(fc2_w, np.float32)
    W12 = np.asarray(W12, np.float32)
    fc4_w = np.asarray(fc4_w, np.float32)
    b1m = np.asarray(bias1, np.float32)                 # [120, 5]
    b2v = np.asarray(bias2, np.float32)[:, 0]

    aw, bw, cw = _host_mats(W11, fc2_w, W12, fc4_w, b1m)
    awb = aw.astype(BF)
    bwb = bw.astype(BF)
    cwb = cw.astype(BF)
    onesb = np.ones((1, 1024), np.float32).astype(BF)

    if "nc" not in _CACHE:
        _CACHE["nc"] = _build_nc()
    nc = _CACHE["nc"]

    in_maps = []
    for c in range(NCORES):
        xs = x[c * BC:(c + 1) * BC]                      # [16384, 40, 10]
        xsr = xs.reshape(NBLK, NB, 400).transpose(2, 0, 1)
        xtc = np.ascontiguousarray(
            xsr.reshape(4, 100, NBLK, NB).transpose(1, 2, 0, 3)
        ).reshape(100, NBLK * 4 * NB).astype(BF)
        in_maps.append({"xt": xtc, "aw": awb, "bw": bwb, "cw": cwb,
                        "ones": onesb})

    res = run_bass_kernel_spmd(nc, in_maps, core_ids=list(range(NCORES)),
                               trace=_trace)
    outs = []
    for c in range(NCORES):
        op = np.asarray(res.results[c]["outp"], np.float32)  # [12, BC]
        if COLTILE:
            outs.append(op.reshape(4, 3, BC).sum(axis=0))
        else:
            outs.append(op.reshape(4, 3, BC)[0])
    full = np.concatenate(outs, axis=1).T + b2v[None, :]
    if _trace:
        kernel.last_exec_time_ns = res.exec_time_ns
    return full.astype(np.float32)


# revision 16
# speedup vs baseline: 1.0582x; 1.0582x over previous
"""Trainium2 Bass kernel for nn_BL_36721970381090 (dense_mlp).

Math: the reference collapses to out[b] = M2 @ relu(M1 @ vec(x[b]) + b1) + b2,
M1 = perm(kron(W11, fc2_w)) [600, 400] (exact rank 200), M2 [3, 600].

Per 512-batch block (u-major intermediate g = 200 rows split 120+80):
  A (8 MMs): pg [128, 1024] psum (2 banks):
     bank0 = g0 (u in {0,1,2}) [120 r], bank1 = g1 (u in {3,4}) [80 r + 40 zero r]
     each bank accumulates 4 chunk-MMs contracting 100 x-rows.
  g-copy (1 op): pg[0:120, 0:1024] -> g_sb bf16. g_sb is a STATIC tile whose
     row 120 holds constant 1.0 (written once) -> bias1 is folded into stage B
     as contraction row 120 of the stationary.
  B (5 MMs): y chunk m = (t, u=m) -> y_super [128, 2560] psum (5 banks,
     chunk m in bank m); lhsT = bw2[0:121, 120m:...] (row 120 = bias1[:, m]).
  relu (1 op): y_super[0:120, 0:2560] -> r_super bf16 (no bias needed).
  C (5 MMs): col-tiled m=0..3 at tile_position (0, 32m) -> concurrent in one
     PE span; partials in pc bank partitions {0-2,32-34,64-66,96-98}; m=4
     accumulates onto the (0,0) slice (start=False).
  pc-copy (1 op) -> osb; 4 partial [3, *] slices summed on host.

Engine economy: 3 ACT/DVE ops per block (vs 12 naive) because engine cost is
free-size driven (~1ns/elem + ~130ns), partitions are free.
"""

import numpy as np
import ml_dtypes
from contextlib import ExitStack

import concourse.bass as bass
import concourse.bacc as bacc
import concourse.mybir as mybir
from concourse.bass import ds
from concourse.tile import TileContext
from concourse.bass_utils import run_bass_kernel_spmd

B, D1, D2 = 131072, 40, 10
T0, T1, O0 = 120, 5, 3
NCORES = 8
BC = B // NCORES          # 16384 batch per core
NB = 512                  # psum free-dim block
NBLK = BC // NB           # 32 blocks
XGRP = 1                  # blocks per x DMA
OGRP = 4                  # blocks per output DMA group

F32 = mybir.dt.float32
BF16 = mybir.dt.bfloat16
BF = ml_dtypes.bfloat16
RELU = mybir.ActivationFunctionType.Relu
COPY = mybir.ActivationFunctionType.Copy
ADD = mybir.AluOpType.add
MAX = mybir.AluOpType.max

COLTILE = True            # col-tiled stage C (4 concurrent MMs)

_CACHE = {}


def _build_nc():
    nc = bacc.Bacc()
    xt = nc.dram_tensor("xt", (100, NBLK * 4 * NB), BF16, kind="ExternalInput")
    aw = nc.dram_tensor("aw", (100, 1024), BF16, kind="ExternalInput")
    bw = nc.dram_tensor("bw", (121, 600), BF16, kind="ExternalInput")
    cw = nc.dram_tensor("cw", (120, 15), BF16, kind="ExternalInput")
    ones = nc.dram_tensor("ones", (1, 1024), BF16, kind="ExternalInput")
    outp = nc.dram_tensor("outp", (12, BC), F32, kind="ExternalOutput")

    with TileContext(nc) as tc, ExitStack() as ctx:
        consts = ctx.enter_context(tc.tile_pool(name="consts", bufs=1))
        aw_sb = consts.tile([100, 1024], BF16, tag="aw")
        bw_sb = consts.tile([121, 600], BF16, tag="bw")
        cw_sb = consts.tile([120, 15], BF16, tag="cw")
        # two static g tiles (separate ACT/DVE writers -> no serialization);
        # row 120 of each holds constant 1.0 (bias-fold contraction row)
        g0_sb = consts.tile([128, NB], BF16, tag="g0sb")
        g1_sb = consts.tile([128, NB], BF16, tag="g1sb")

        xpool = ctx.enter_context(tc.tile_pool(name="xp", bufs=3))
        rpool = ctx.enter_context(tc.tile_pool(name="rp", bufs=2))
        opool = ctx.enter_context(tc.tile_pool(name="op", bufs=2))
        pgpool = ctx.enter_context(tc.tile_pool(name="pg", bufs=1, space="PSUM"))
        pypool = ctx.enter_context(tc.tile_pool(name="py", bufs=1, space="PSUM"))
        pcpool = ctx.enter_context(tc.tile_pool(name="pc", bufs=1, space="PSUM"))

        xts = {}

        def issue_x_dma(jj):
            if jj % XGRP == 0 and jj < NBLK:
                xt2 = xpool.tile([100, XGRP * 4 * NB], BF16, tag="xt2",
                                 name=f"xt2_{jj}")
                nc.sync.dma_start(xt2[:, :],
                                  xt[:, ds(jj * 4 * NB, XGRP * 4 * NB)])
                xts[jj // XGRP] = xt2

        # x DMA first (A(0) critical path), then the small consts
        issue_x_dma(0)
        nc.sync.dma_start(aw_sb[:, :], aw[:, :])
        issue_x_dma(1)
        nc.sync.dma_start(bw_sb[:, :], bw[:, :])
        nc.sync.dma_start(cw_sb[:, :], cw[:, :])
        nc.sync.dma_start(g0_sb[120:121, :], ones[:, ds(0, NB)])
        nc.sync.dma_start(g1_sb[120:121, :], ones[:, ds(NB, NB)])

        def issue_A(jj):
            """DMA prefetch + 8 A-matmuls into a fresh pg tile. Returns pg."""
            issue_x_dma(jj + 2)
            xt2 = xts[jj // XGRP]
            xoff = (jj % XGRP) * 4 * NB
            pg = pgpool.tile([128, 2 * NB], F32, tag="pg", name=f"pg_{jj}")
            # g0 bank first so gcopy-a can fire mid-A
            for c in range(4):
                nc.tensor.matmul(pg[0:120, ds(0, NB)],
                                 aw_sb[:, ds(256 * c, 120)],
                                 xt2[:, ds(xoff + NB * c, NB)],
                                 start=(c == 0), stop=(c == 3))
            for c in range(4):
                nc.tensor.matmul(pg[0:120, ds(NB, NB)],
                                 aw_sb[:, ds(256 * c + 128, 120)],
                                 xt2[:, ds(xoff + NB * c, NB)],
                                 start=(c == 0), stop=(c == 3))
            return pg

        pg = issue_A(0)
        for jj in range(NBLK):
            # g-copies: bank0 on ACT, bank1 on DVE (separate dest tiles ->
            # independent writers, no serialization)
            nc.scalar.activation(g0_sb[0:120, :], pg[0:120, ds(0, NB)], COPY)
            nc.vector.tensor_copy(g1_sb[0:120, :], pg[0:120, ds(NB, NB)])

            # --- stage B: y_super [128, 2560] (chunk m in bank m) ---
            ys = pypool.tile([128, 5 * NB], F32, tag="ys", name=f"ys_{jj}")
            for m in range(5):
                gsb = g0_sb if m < 3 else g1_sb
                nc.tensor.matmul(ys[0:120, ds(NB * m, NB)],
                                 bw_sb[0:121, ds(120 * m, 120)],
                                 gsb[0:121, :],
                                 start=True, stop=True)
            # relu in 3 parallel ops: ACT chunks 0-1, DVE chunks 2-3,
            # ACT chunk 4 (short critical path before C)
            r_a = rpool.tile([128, 2 * NB], BF16, tag="ra", name=f"ra_{jj}")
            r_b = rpool.tile([128, 2 * NB], BF16, tag="rb", name=f"rb_{jj}")
            r_c = rpool.tile([128, NB], BF16, tag="rc", name=f"rc_{jj}")
            nc.scalar.activation(r_a[0:120, :], ys[0:120, ds(0, 2 * NB)], RELU)
            nc.vector.tensor_scalar(r_b[0:120, :], ys[0:120, ds(2 * NB, 2 * NB)],
                                    0.0, None, op0=MAX)
            nc.scalar.activation(r_c[0:120, :], ys[0:120, ds(4 * NB, NB)], RELU)

            # next block's A-matmuls run while relu(jj) drains
            if jj + 1 < NBLK:
                pg = issue_A(jj + 1)

            # --- stage C ---
            pc = pcpool.tile([128, NB], F32, tag="pc", name=f"pc_{jj}")
            def r_src(m):
                if m < 2:
                    return r_a[0:120, ds(NB * m, NB)]
                if m < 4:
                    return r_b[0:120, ds(NB * (m - 2), NB)]
                return r_c[0:120, :]

            if COLTILE:
                for m in range(4):
                    nc.tensor.matmul(pc[32 * m:32 * m + 3, :],
                                     cw_sb[:, ds(3 * m, 3)], r_src(m),
                                     start=True, stop=(m != 0),
                                     tile_position=(0, 32 * m),
                                     skip_group_check=True)
                nc.tensor.matmul(pc[0:3, :], cw_sb[:, ds(12, 3)], r_src(4),
                                 start=False, stop=True, tile_position=(0, 0),
                                 skip_group_check=True)
            else:
                for m in range(5):
                    nc.tensor.matmul(pc[0:3, :], cw_sb[:, ds(3 * m, 3)],
                                     r_src(m), start=(m == 0), stop=(m == 4))

            # --- drain C partials ---
            if jj % OGRP == 0:
                osb = opool.tile([128, OGRP * NB], F32, tag="osb",
                                 name=f"osb_{jj}")
            oslot = (jj % OGRP) * NB
            if COLTILE:
                nc.vector.tensor_copy(osb[0:99, ds(oslot, NB)], pc[0:99, :])
            else:
                nc.vector.tensor_copy(osb[0:3, ds(oslot, NB)], pc[0:3, :])
            if jj % OGRP == OGRP - 1:
                grp = jj // OGRP
                gslice = ds(grp * OGRP * NB, OGRP * NB)
                if COLTILE:
                    for m in range(4):
                        nc.sync.dma_start(outp[ds(3 * m, 3), gslice],
                                          osb[32 * m:32 * m + 3, :])
                else:
                    nc.sync.dma_start(outp[ds(0, 3), gslice], osb[0:3, :])
    nc.finalize()
    return nc


def _host_mats(W11, fc2_w, W12, fc4_w, bias1):
    """Build aw [100,1024], bw [121,600], cw [120,15] (fp32)."""
    aw = np.zeros((100, 1024), np.float32)
    for c in range(4):
        for p in range(100):
            d = 10 * c + p // 10
            s = p % 10
            for u in range(3):
                aw[p, 256 * c + u * 40 + d] = fc2_w[u, s]
            for u in (3, 4):
                aw[p, 256 * c + 128 + (u - 3) * 40 + d] = fc2_w[u, s]
    bw = np.zeros((121, 600), np.float32)
    for m in range(5):
        if m < 3:
            rows = slice(40 * m, 40 * m + 40)
        else:
            rows = slice(40 * (m - 3), 40 * (m - 3) + 40)
        bw[rows, 120 * m:120 * m + 120] = W11.T        # [40 d, 120 t]
        bw[120, 120 * m:120 * m + 120] = bias1[:, m]   # folded bias1
    M2 = np.kron(W12, fc4_w)                            # [3, 600]
    cw = np.zeros((120, 15), np.float32)
    for m in range(5):
        cw[:, 3 * m:3 * m + 3] = M2[:, m::5].T
    return aw, bw, cw


def kernel(x, W11, fc2_w, bias1, W12, fc4_w, bias2, _trace=False):
    x = np.asarray(x, dtype=np.float32)
    W11 = np.asarray(W11, np.float32)
    fc2_w = np.asarray(fc2_w, np.float32)
    W12 = np.asarray(W12, np.float32)
    fc4_w = np.asarray(fc4_w, np.float32)
    b1m = np.asarray(bias1, np.float32)                 # [120, 5]
    b2v = np.asarray(bias2, np.float32)[:, 0]

    aw, bw, cw = _host_mats(W11, fc2_w, W12, fc4_w, b1m)
    awb = aw.astype(BF)
    bwb = bw.astype(BF)
    cwb = cw.astype(BF)
    onesb = np.ones((1, 1024), np.float32).astype(BF)

    if "nc" not in _CACHE:
        _CACHE["nc"] = _build_nc()
    nc = _CACHE["nc"]

    in_maps = []
    for c in range(NCORES):
        xs = x[c * BC:(c + 1) * BC]                      # [16384, 40, 10]
        xsr = xs.reshape(NBLK, NB, 400).transpose(2, 0, 1)
        xtc = np.ascontiguousarray(
            xsr.reshape(4, 100, NBLK, NB).transpose(1, 2, 0, 3)
        ).reshape(100, NBLK * 4 * NB).astype(BF)
        in_maps.append({"xt": xtc, "aw": awb, "bw": bwb, "cw": cwb,
                        "ones": onesb})

    res = run_bass_kernel_spmd(nc, in_maps, core_ids=list(range(NCORES)),
                               trace=_trace)
    outs = []
    for c in range(NCORES):
        op = np.asarray(res.results[c]["outp"], np.float32)  # [12, BC]
        if COLTILE:
            outs.append(op.reshape(4, 3, BC).sum(axis=0))
        else:
            outs.append(op.reshape(4, 3, BC)[0])
    full = np.concatenate(outs, axis=1).T + b2v[None, :]
    if _trace:
        kernel.last_exec_time_ns = res.exec_time_ns
    return full.astype(np.float32)


# revision 17
# speedup vs baseline: 1.0867x; 1.0269x over previous
"""Trainium2 Bass kernel for nn_BL_36721970381090 (dense_mlp).

Math: the reference collapses to out[b] = M2 @ relu(M1 @ vec(x[b]) + b1) + b2,
M1 = perm(kron(W11, fc2_w)) [600, 400] (exact rank 200), M2 [3, 600].

Per 512-batch block (u-major intermediate g = 200 rows split 120+80):
  A (8 MMs): pg [128, 1024] psum (2 banks):
     bank0 = g0 (u in {0,1,2}) [120 r], bank1 = g1 (u in {3,4}) [80 r + 40 zero r]
     each bank accumulates 4 chunk-MMs contracting 100 x-rows.
  g-copy (1 op): pg[0:120, 0:1024] -> g_sb bf16. g_sb is a STATIC tile whose
     row 120 holds constant 1.0 (written once) -> bias1 is folded into stage B
     as contraction row 120 of the stationary.
  B (5 MMs): y chunk m = (t, u=m) -> y_super [128, 2560] psum (5 banks,
     chunk m in bank m); lhsT = bw2[0:121, 120m:...] (row 120 = bias1[:, m]).
  relu (1 op): y_super[0:120, 0:2560] -> r_super bf16 (no bias needed).
  C (5 MMs): col-tiled m=0..3 at tile_position (0, 32m) -> concurrent in one
     PE span; partials in pc bank partitions {0-2,32-34,64-66,96-98}; m=4
     accumulates onto the (0,0) slice (start=False).
  pc-copy (1 op) -> osb; 4 partial [3, *] slices summed on host.

Engine economy: 3 ACT/DVE ops per block (vs 12 naive) because engine cost is
free-size driven (~1ns/elem + ~130ns), partitions are free.
"""

import numpy as np
import ml_dtypes
from contextlib import ExitStack

import concourse.bass as bass
import concourse.bacc as bacc
import concourse.mybir as mybir
from concourse.bass import ds
from concourse.tile import TileContext
from concourse.bass_utils import run_bass_kernel_spmd

B, D1, D2 = 131072, 40, 10
T0, T1, O0 = 120, 5, 3
NCORES = 8
BC = B // NCORES          # 16384 batch per core
NB = 512                  # psum free-dim block
NBLK = BC // NB           # 32 blocks
XGRP = 1                  # blocks per x DMA
OGRP = 4                  # blocks per output DMA group

F32 = mybir.dt.float32
BF16 = mybir.dt.bfloat16
BF = ml_dtypes.bfloat16
RELU = mybir.ActivationFunctionType.Relu
COPY = mybir.ActivationFunctionType.Copy
ADD = mybir.AluOpType.add
MAX = mybir.AluOpType.max

COLTILE = True            # col-tiled stage C (4 concurrent MMs)

_CACHE = {}


def _build_nc():
    nc = bacc.Bacc()
    xt = nc.dram_tensor("xt", (100, NBLK * 4 * NB), BF16, kind="ExternalInput")
    aw = nc.dram_tensor("aw", (100, 1024), BF16, kind="ExternalInput")
    bw = nc.dram_tensor("bw", (121, 600), BF16, kind="ExternalInput")
    cw = nc.dram_tensor("cw", (120, 15), BF16, kind="ExternalInput")
    ones = nc.dram_tensor("ones", (1, 1024), BF16, kind="ExternalInput")
    outp = nc.dram_tensor("outp", (12, BC), F32, kind="ExternalOutput")

    with TileContext(nc) as tc, ExitStack() as ctx:
        consts = ctx.enter_context(tc.tile_pool(name="consts", bufs=1))
        aw_sb = consts.tile([100, 1024], BF16, tag="aw")
        bw_sb = consts.tile([121, 600], BF16, tag="bw")
        cw_sb = consts.tile([120, 15], BF16, tag="cw")
        # two static g tiles (separate ACT/DVE writers -> no serialization);
        # row 120 of each holds constant 1.0 (bias-fold contraction row)
        g0_sb = consts.tile([128, NB], BF16, tag="g0sb")
        g1_sb = consts.tile([128, NB], BF16, tag="g1sb")

        xpool = ctx.enter_context(tc.tile_pool(name="xp", bufs=3))
        rpool = ctx.enter_context(tc.tile_pool(name="rp", bufs=2))
        opool = ctx.enter_context(tc.tile_pool(name="op", bufs=2))
        pgpool = ctx.enter_context(tc.tile_pool(name="pg", bufs=1, space="PSUM"))
        pypool = ctx.enter_context(tc.tile_pool(name="py", bufs=1, space="PSUM"))
        pcpool = ctx.enter_context(tc.tile_pool(name="pc", bufs=1, space="PSUM"))

        xts = {}

        def issue_x_dma(jj):
            if jj % XGRP == 0 and jj < NBLK:
                xt2 = xpool.tile([100, XGRP * 4 * NB], BF16, tag="xt2",
                                 name=f"xt2_{jj}")
                nc.sync.dma_start(xt2[:, :],
                                  xt[:, ds(jj * 4 * NB, XGRP * 4 * NB)])
                xts[jj // XGRP] = xt2

        # x DMA first (A(0) critical path), then the small consts
        issue_x_dma(0)
        nc.sync.dma_start(aw_sb[:, :], aw[:, :])
        issue_x_dma(1)
        nc.sync.dma_start(bw_sb[:, :], bw[:, :])
        nc.sync.dma_start(cw_sb[:, :], cw[:, :])
        nc.sync.dma_start(g0_sb[120:121, :], ones[:, ds(0, NB)])
        nc.sync.dma_start(g1_sb[120:121, :], ones[:, ds(NB, NB)])

        def issue_A(jj):
            """DMA prefetch + 8 A-matmuls into a fresh pg tile. Returns pg."""
            issue_x_dma(jj + 2)
            xt2 = xts[jj // XGRP]
            xoff = (jj % XGRP) * 4 * NB
            pg = pgpool.tile([128, 2 * NB], F32, tag="pg", name=f"pg_{jj}")
            # g0 bank first so gcopy-a can fire mid-A
            for c in range(4):
                nc.tensor.matmul(pg[0:120, ds(0, NB)],
                                 aw_sb[:, ds(256 * c, 120)],
                                 xt2[:, ds(xoff + NB * c, NB)],
                                 start=(c == 0), stop=(c == 3))
            for c in range(4):
                nc.tensor.matmul(pg[0:120, ds(NB, NB)],
                                 aw_sb[:, ds(256 * c + 128, 120)],
                                 xt2[:, ds(xoff + NB * c, NB)],
                                 start=(c == 0), stop=(c == 3))
            return pg

        rtiles = {}
        state = {"osb": None}

        def issue_gcopy(pg):
            # bank0 on ACT, bank1 on DVE; separate dest tiles -> parallel
            nc.scalar.activation(g0_sb[0:120, :], pg[0:120, ds(0, NB)], COPY)
            nc.vector.tensor_copy(g1_sb[0:120, :], pg[0:120, ds(NB, NB)])

        def issue_C(jc):
            """Stage C for block jc (relu outputs are a full cycle old)."""
            r_a, r_b, r_c = rtiles.pop(jc)

            def r_src(m):
                if m < 2:
                    return r_a[0:120, ds(NB * m, NB)]
                if m < 4:
                    return r_b[0:120, ds(NB * (m - 2), NB)]
                return r_c[0:120, :]

            pc = pcpool.tile([128, NB], F32, tag="pc", name=f"pc_{jc}")
            if COLTILE:
                for m in range(4):
                    nc.tensor.matmul(pc[32 * m:32 * m + 3, :],
                                     cw_sb[:, ds(3 * m, 3)], r_src(m),
                                     start=True, stop=(m != 0),
                                     tile_position=(0, 32 * m),
                                     skip_group_check=True)
                nc.tensor.matmul(pc[0:3, :], cw_sb[:, ds(12, 3)], r_src(4),
                                 start=False, stop=True, tile_position=(0, 0),
                                 skip_group_check=True)
            else:
                for m in range(5):
                    nc.tensor.matmul(pc[0:3, :], cw_sb[:, ds(3 * m, 3)],
                                     r_src(m), start=(m == 0), stop=(m == 4))
            # drain partials
            if jc % OGRP == 0:
                state["osb"] = opool.tile([128, OGRP * NB], F32, tag="osb",
                                          name=f"osb_{jc}")
            osb = state["osb"]
            oslot = (jc % OGRP) * NB
            rows = 99 if COLTILE else 3
            nc.vector.tensor_copy(osb[0:rows, ds(oslot, NB)], pc[0:rows, :])
            if jc % OGRP == OGRP - 1:
                gslice = ds((jc // OGRP) * OGRP * NB, OGRP * NB)
                if COLTILE:
                    for m in range(4):
                        nc.sync.dma_start(outp[ds(3 * m, 3), gslice],
                                          osb[32 * m:32 * m + 3, :])
                else:
                    nc.sync.dma_start(outp[ds(0, 3), gslice], osb[0:3, :])

        pg = issue_A(0)
        issue_gcopy(pg)
        for jj in range(NBLK):
            # --- stage B: y_super [128, 2560] (chunk m in bank m) ---
            ys = pypool.tile([128, 5 * NB], F32, tag="ys", name=f"ys_{jj}")
            for m in range(5):
                gsb = g0_sb if m < 3 else g1_sb
                nc.tensor.matmul(ys[0:120, ds(NB * m, NB)],
                                 bw_sb[0:121, ds(120 * m, 120)],
                                 gsb[0:121, :],
                                 start=True, stop=True)
            # relu in 3 parallel ops: ACT chunks 0-1, DVE 2-3, ACT 4
            r_a = rpool.tile([128, 2 * NB], BF16, tag="ra", name=f"ra_{jj}")
            r_b = rpool.tile([128, 2 * NB], BF16, tag="rb", name=f"rb_{jj}")
            r_c = rpool.tile([128, NB], BF16, tag="rc", name=f"rc_{jj}")
            nc.scalar.activation(r_a[0:120, :], ys[0:120, ds(0, 2 * NB)], RELU)
            nc.vector.tensor_scalar(r_b[0:120, :], ys[0:120, ds(2 * NB, 2 * NB)],
                                    0.0, None, op0=MAX)
            nc.scalar.activation(r_c[0:120, :], ys[0:120, ds(4 * NB, NB)], RELU)
            rtiles[jj] = (r_a, r_b, r_c)

            # next block's A-matmuls + g-copies run while relu(jj) drains
            if jj + 1 < NBLK:
                pg = issue_A(jj + 1)
                issue_gcopy(pg)

            # stage C for the PREVIOUS block: its relus completed a full
            # cycle ago, so the C matmuls never stall the PE
            if jj >= 1:
                issue_C(jj - 1)
        issue_C(NBLK - 1)
    nc.finalize()
    return nc


def _host_mats(W11, fc2_w, W12, fc4_w, bias1):
    """Build aw [100,1024], bw [121,600], cw [120,15] (fp32)."""
    aw = np.zeros((100, 1024), np.float32)
    for c in range(4):
        for p in range(100):
            d = 10 * c + p // 10
            s = p % 10
            for u in range(3):
                aw[p, 256 * c + u * 40 + d] = fc2_w[u, s]
            for u in (3, 4):
                aw[p, 256 * c + 128 + (u - 3) * 40 + d] = fc2_w[u, s]
    bw = np.zeros((121, 600), np.float32)
    for m in range(5):
        if m < 3:
            rows = slice(40 * m, 40 * m + 40)
        else:
            rows = slice(40 * (m - 3), 40 * (m - 3) + 40)
        bw[rows, 120 * m:120 * m + 120] = W11.T        # [40 d, 120 t]
        bw[120, 120 * m:120 * m + 120] = bias1[:, m]   # folded bias1
    M2 = np.kron(W12, fc4_w)                            # [3, 600]
    cw = np.zeros((120, 15), np.float32)
    for m in range(5):
        cw[:, 3 * m:3 * m + 3] = M2[:, m::5].T
    return aw, bw, cw


def kernel(x, W11, fc2_w, bias1, W12, fc4_w, bias2, _trace=False):
    x = np.asarray(x, dtype=np.float32)
    W11 = np.asarray(W11, np.float32)
    fc2_w = np.asarray(fc2_w, np.float32)
    W12 = np.asarray(W12, np.float32)
    fc4_w = np.asarray(fc4_w, np.float32)
    b1m = np.asarray(bias1, np.float32)                 # [120, 5]
    b2v = np.asarray(bias2, np.float32)[:, 0]

    aw, bw, cw = _host_mats(W11, fc2_w, W12, fc4_w, b1m)
    awb = aw.astype(BF)
    bwb = bw.astype(BF)
    cwb = cw.astype(BF)
    onesb = np.ones((1, 1024), np.float32).astype(BF)

    if "nc" not in _CACHE:
        _CACHE["nc"] = _build_nc()
    nc = _CACHE["nc"]

    in_maps = []
    for c in range(NCORES):
        xs = x[c * BC:(c + 1) * BC]                      # [16384, 40, 10]
        xsr = xs.reshape(NBLK, NB, 400).transpose(2, 0, 1)
        xtc = np.ascontiguousarray(
            xsr.reshape(4, 100, NBLK, NB).transpose(1, 2, 0, 3)
        ).reshape(100, NBLK * 4 * NB).astype(BF)
        in_maps.append({"xt": xtc, "aw": awb, "bw": bwb, "cw": cwb,
                        "ones": onesb})

    res = run_bass_kernel_spmd(nc, in_maps, core_ids=list(range(NCORES)),
                               trace=_trace)
    outs = []
    for c in range(NCORES):
        op = np.asarray(res.results[c]["outp"], np.float32)  # [12, BC]
        if COLTILE:
            outs.append(op.reshape(4, 3, BC).sum(axis=0))
        else:
            outs.append(op.reshape(4, 3, BC)[0])
    full = np.concatenate(outs, axis=1).T + b2v[None, :]
    if _trace:
        kernel.last_exec_time_ns = res.exec_time_ns
    return full.astype(np.float32)
